# revision 36
# baseline (speedup 1.0000x reference)
"""GQA kernel builder for TRN2 (8-core tensor-parallel over heads).

Per core: 4 Q heads (128-dim each) + the 1 KV head they share.
All activations are kept feature-major ([feat_part, seq_free]) so every
matmul contracts over the partition dim:

  kT[d, t]  = sum_e WkT[e, d] * hT[e, t]          (lhsT=WkT tile, rhs=hT tile)
  V[t, d]   = sum_e hT[e, t] * WvT[e, d]          (lhsT=hT tile, rhs=WvT tile)
  qT[d, s]  = sum_e WqT[e, d] * hT[e, s]          (lhsT=WqT tile, rhs=hT tile)
  S^T[t, s] = sum_d kT[d, t] * qT[d, s]           (single matmul, d=128)
  P^T       = exp(S^T / sqrt(128))                (ScalarE, no max-subtract:
                                                   |scores| <~ 8 here)
  O^T[d, s] = sum_t V[t, d] * P^T[t, s]           (accumulate over t chunks)
  den[s]    = sum_t P^T[t, s]                     (ones-matmul, M=128 so the
                                                   sum lands broadcast)
  ao^T      = O^T * (1/den)                       (DVE reciprocal + mul)
  outT[o,s] = sum_f WoT[f, o] * ao^T[f, s]        (partial; host sums cores)

Structure (all bf16, f32 PSUM accumulate; ~468us HW, ~4x over the
f32 per-phase baseline):
- Few, large multi-dim DMAs (a dma_start costs ~0.6us of serial SP
  sequencer issue time), ordered by consumption so the first matmuls
  start ~15us in.
- Phase A streams hT once (2-deep s-tile double buffer): K, V, Q per
  s-tile. Q(st3) is held back; its hT tile persists.
- Phase B runs st-outer and interleaves PE filler between the exp
  (ScalarE) and the AV matmuls of each iteration: phase-C o_proj
  blocks of the previous s-tile (or the held-back Q(st3) blocks for
  st0). The softmax denominator is quad-summed on DVE and
  partition-reduced by 4 accumulating ones-matmuls.
- o_proj partials are written as bf16 and summed on the host.
"""

import math
import numpy as np
from contextlib import ExitStack

import concourse.bass as bass
import concourse.mybir as mybir
import concourse.tile as tile
from concourse.vector_clock import ScopedClock

F32 = mybir.dt.float32
BF16 = mybir.dt.bfloat16

S = 2048
E = 4096
HD = 128
H = 4          # Q heads per core
DQ = H * HD    # 512
ST = 512       # seq tile (free dim of most matmuls)
NST = S // ST  # 4
NE = E // 128  # 32
NT = S // 128  # 16
SCALE = 1.0 / math.sqrt(128.0)

MAX_DRAIN_WAITS = 1


class SplitDrainTileContext(tile.TileContext):
    """Walrus CoreV3 rejects >1 sync wait on an instruction; TileContext's
    exit attaches the whole residual vector clock to one Drain. Split it
    into a chain of Drains (SP executes them in order — equivalent)."""

    def _drain_and_barrier(self, tick_clock, wait_clock):
        drain_inst = self.nc.sync.drain()
        wait_clock.add_sem_waits(
            drain_inst.ins, ScopedClock({None: tick_clock.global_clock})
        )
        si = drain_inst.ins.sync_info
        waits = list(si.on_wait) if si is not None and si.on_wait else []
        if len(waits) > MAX_DRAIN_WAITS:
            si.on_wait = waits[:MAX_DRAIN_WAITS]
            rest = waits[MAX_DRAIN_WAITS:]
            for i in range(0, len(rest), MAX_DRAIN_WAITS):
                extra = self.nc.sync.drain()
                extra.ins.sync_info = mybir.SyncInfo(
                    on_wait=rest[i : i + MAX_DRAIN_WAITS], on_update=[]
                )

        self.nc.all_engine_barrier()
        assert self.sems is not None
        popped = self.nc._tile_sem_poison_stack.pop()
        assert popped is self._sem_poison
        self.nc.clear_and_free_semaphores(list(self.sems.allocated().values()))
        self.nc.all_engine_barrier()


def _split_multi_wait_insts(nc, max_waits: int = 1):
    """Walrus CoreV2/V3 codegen rejects instructions with more than one sync
    wait command. Hoist excess waits onto preceding same-engine NoOps — the
    engine executes them in order, so the gating is equivalent (for DMA the
    issuing sequencer stalls instead of the DGE queue: conservative, safe)."""
    for fn in nc.m.functions:
        for blk in fn.blocks:
            out = []
            for inst in blk.instructions:
                si = inst.sync_info
                waits = list(si.on_wait) if si is not None and si.on_wait else []
                if len(waits) > max_waits:
                    excess, keep = waits[:-max_waits], waits[-max_waits:]
                    for j, w in enumerate(excess):
                        nop = mybir.InstNoOp(name=f"{inst.name}-sw{j}")
                        nop.engine = inst.engine
                        nop.sync_info = mybir.SyncInfo(on_wait=[w], on_update=[])
                        out.append(nop)
                    si.on_wait = keep
                out.append(inst)
            blk.instructions = out


def build(dtype_mode: str = "bf16") -> bass.Bass:
    """dtype_mode: 'f32' | 'bf16'"""
    D = BF16 if dtype_mode == "bf16" else F32
    OUT_D = BF16 if dtype_mode == "bf16" else F32

    nc = bass.Bass()
    # All DRAM params are host-pre-tiled to the exact SBUF layout so every
    # DMA reads/writes large contiguous per-partition lines (the flat [E,S]
    # layout only reached ~175GB/s on the strided 1KB-line loads).
    hT4 = nc.declare_dram_parameter("hT4", [NST, 128, NE, ST], D, isOutput=False)
    wq4 = nc.declare_dram_parameter("wq4", [128, NE, DQ], D, isOutput=False)
    wk4 = nc.declare_dram_parameter("wk4", [128, NE, HD], D, isOutput=False)
    wv4 = nc.declare_dram_parameter("wv4", [128, NE, HD], D, isOutput=False)
    wo4 = nc.declare_dram_parameter("wo4", [128, H, E], D, isOutput=False)
    out4 = nc.declare_dram_parameter(
        "out4", [E // 128, NST, 128, ST], OUT_D, isOutput=True
    )

    with SplitDrainTileContext(nc) as tc, ExitStack() as octx:
        persist = octx.enter_context(tc.tile_pool(name="persist", bufs=1))
        weights = octx.enter_context(tc.tile_pool(name="weights", bufs=1))
        h3_pool = octx.enter_context(tc.tile_pool(name="h3", bufs=1))
        ps_mm = octx.enter_context(tc.tile_pool(name="ps_mm", bufs=5, space="PSUM"))
        ps_acc = octx.enter_context(tc.tile_pool(name="ps_acc", bufs=2, space="PSUM"))
        ps_den = octx.enter_context(tc.tile_pool(name="ps_den", bufs=1, space="PSUM"))

        # Persistent activations (merged into few tiles: each tile slot costs
        # semaphores that are individually zeroed in the NEFF epilogue)
        qT_all = persist.tile([128, H, S], D, name="qT", tag="qT")
        kT_t = persist.tile([128, S], D, name="kT", tag="kT")
        V_all = persist.tile([128, NT, HD], D, name="V", tag="V")
        # All-ones stationary operand: the denominator matmul uses M=128 so
        # the row-sum lands broadcast across all 128 PSUM partitions (same
        # N-cycle streaming cost as M=1, and DVE can then consume it without
        # a partition-broadcast).
        ones = persist.tile([128, 128], D, name="ones", tag="ones")
        nc.vector.memset(ones[:], 1.0)
        # Warm the Exp+Ln activation table during the DMA-bound head: the
        # lazy ACT_TABLE_LOAD otherwise costs ~1.3us at the first phase-B
        # exp. Using Ln here too pins the shared natural_log_exp_and_others
        # table (the denominator reciprocal runs as exp(-ln(x)) on ScalarE).
        warm = persist.tile([128, 1], D, name="warm", tag="warm")
        nc.scalar.activation(
            warm[:], ones[:, :1], mybir.ActivationFunctionType.Ln, scale=1.0
        )
        nc.scalar.activation(
            warm[:], ones[:, :1], mybir.ActivationFunctionType.Exp, scale=SCALE
        )

        wq_all = weights.tile([128, NE, DQ], D, name="wq", tag="wq")
        # h for the last s-tile stays resident past phase A: its Q blocks
        # are held back to fill the PE during B(st0) (which has no phase-C
        # filler), so the tile must outlive the phase-A h pool.
        h3_all = h3_pool.tile([128, NE, ST], D, name="h3", tag="h3")

        def q_block(dq, st, h_src):
            ssl = slice(st * ST, (st + 1) * ST)
            ps = ps_mm.tile([128, ST], F32, name="mm", tag="mm")
            for e in range(NE):
                nc.tensor.matmul(
                    ps[:],
                    wq_all[:, e, dq * 128 : (dq + 1) * 128],
                    h_src[:, e, :],
                    start=(e == 0), stop=(e == NE - 1),
                )
            nc.vector.tensor_copy(qT_all[:, dq, ssl], ps[:])

        # ---- Phase A (fused): one pass over hT computes K, V, Q.
        # DMAs are few and big (each dma_start costs ~0.6us of serial issue
        # time on the SP sequencer) and issued in consumption order:
        # wk -> hT(st0) -> wv -> hT(st1) -> wq -> hT(st2) -> hT(st3).
        with ExitStack() as actx:
            wkv_pool = actx.enter_context(tc.tile_pool(name="wkv", bufs=1))
            hA_pool = actx.enter_context(tc.tile_pool(name="hA", bufs=2))
            wk_all = wkv_pool.tile([128, NE, HD], D, name="wk", tag="wk")
            wv_all = wkv_pool.tile([128, NE, HD], D, name="wv", tag="wv")
            # Single serial DMA stream on Sync in exact consumption order:
            # splitting across the two HWDGE queues was tried and lost —
            # concurrent queues split the (slow, ~200GB/s early-window)
            # bandwidth and the late consumers' loads finish LATER than a
            # serial consumption-ordered stream.
            nc.sync.dma_start(wk_all[:, : NE // 8, :], wk4[:, : NE // 8, :])
            for st in range(NST):
                ssl = slice(st * ST, (st + 1) * ST)
                if st == NST - 1:
                    h_st = h3_all
                else:
                    h_st = hA_pool.tile([128, NE, ST], D, name="h", tag="h")
                if st == 0:
                    nc.sync.dma_start(
                        h_st[:, : NE // 8, :], hT4[0, :, : NE // 8, :]
                    )
                    nc.sync.dma_start(wk_all[:, NE // 8 :, :], wk4[:, NE // 8 :, :])
                    nc.sync.dma_start(
                        h_st[:, NE // 8 : NE // 4, :],
                        hT4[0, :, NE // 8 : NE // 4, :],
                    )
                    for qtr in range(1, 4):
                        nc.sync.dma_start(
                            h_st[:, qtr * (NE // 4) : (qtr + 1) * (NE // 4), :],
                            hT4[0, :, qtr * (NE // 4) : (qtr + 1) * (NE // 4), :],
                        )
                        if qtr == 1:
                            nc.sync.dma_start(wv_all[:], wv4[:, :, :])
                    nc.sync.dma_start(
                        wq_all[:, :, : DQ // 2], wq4[:, :, : DQ // 2]
                    )
                    nc.sync.dma_start(
                        wq_all[:, :, DQ // 2 :], wq4[:, :, DQ // 2 :]
                    )
                else:
                    nc.sync.dma_start(h_st[:], hT4[st, :, :, :])
                # K projection
                ps = ps_mm.tile([128, ST], F32, name="mm", tag="mm")
                for e in range(NE):
                    nc.tensor.matmul(
                        ps[:], wk_all[:, e, :], h_st[:, e, :],
                        start=(e == 0), stop=(e == NE - 1),
                    )
                nc.vector.tensor_copy(kT_t[:, ssl], ps[:])
                # V projection (natural [t, d] layout)
                for tc4 in range(ST // 128):
                    tglob = st * (ST // 128) + tc4
                    ps = ps_mm.tile([128, HD], F32, name="mm", tag="mm")
                    for e in range(NE):
                        nc.tensor.matmul(
                            ps[:],
                            h_st[:, e, tc4 * 128 : (tc4 + 1) * 128],
                            wv_all[:, e, :],
                            start=(e == 0), stop=(e == NE - 1),
                        )
                    nc.vector.tensor_copy(V_all[:, tglob, :], ps[:])
                # Q projection (st3 blocks held back as B(st0) filler)
                if st < NST - 1:
                    for dq in range(H):
                        q_block(dq, st, h_st)

        # ---- Phases B+C interleaved: B runs st-outer so phase-C blocks of
        # the previous s-tile can fill the PE while ScalarE runs the exps.
        # The softmax denominator is quad-summed on DVE (12 adds) and
        # partition-reduced by 4 accumulating ones-matmuls (PE cost ~0.9us
        # per iteration instead of 3.5us for 16 ones-matmuls).
        with ExitStack() as bctx:
            pt_pool = bctx.enter_context(tc.tile_pool(name="pt", bufs=20))
            dsum_pool = bctx.enter_context(tc.tile_pool(name="dsum", bufs=10))
            nrm_pool = bctx.enter_context(tc.tile_pool(name="nrm", bufs=2))
            ao_pool = bctx.enter_context(tc.tile_pool(name="ao", bufs=1))
            stg_pool = bctx.enter_context(tc.tile_pool(name="stg", bufs=6))
            wo_all = weights.tile([128, H, E], D, name="wo", tag="wo")
            aoT_all = ao_pool.tile([128, H, S], D, name="ao", tag="ao")

            def c_block(oc, st):
                ssl = slice(st * ST, (st + 1) * ST)
                ps = ps_mm.tile([128, ST], F32, name="mm", tag="mm")
                for fc in range(H):
                    nc.tensor.matmul(
                        ps[:],
                        wo_all[:, fc, oc * 128 : (oc + 1) * 128],
                        aoT_all[:, fc, ssl],
                        start=(fc == 0), stop=(fc == H - 1),
                    )
                stg = stg_pool.tile([128, ST], OUT_D, name="stg", tag="stg")
                nc.vector.tensor_copy(stg[:], ps[:])
                nc.sync.dma_start(out4[oc, st, :, :], stg[:])

            for st in range(NST):
                ssl = slice(st * ST, (st + 1) * ST)
                for h in range(H):
                    # scores + exp
                    pt_tiles = []
                    for tcn in range(NT):
                        ps = ps_mm.tile([128, ST], F32, name="mm", tag="mm")
                        nc.tensor.matmul(
                            ps[:],
                            kT_t[:, tcn * 128 : (tcn + 1) * 128],
                            qT_all[:, h, ssl],
                            start=True, stop=True,
                        )
                        pt = pt_pool.tile([128, ST], D, name="pt", tag="pt")
                        nc.scalar.activation(
                            pt[:], ps[:], mybir.ActivationFunctionType.Exp,
                            scale=SCALE,
                        )
                        pt_tiles.append(pt)
                    if st == 0 and h == 0:
                        nc.sync.dma_start(wo_all[:], wo4[:, :, :])
                    # filler to keep the PE busy while this iteration's
                    # exps run on ScalarE: phase-C blocks of the previous
                    # s-tile, or (for st0) the held-back Q(st3) blocks
                    if st > 0:
                        for oc in range(h * 8, h * 8 + 8):
                            c_block(oc, st - 1)
                    else:
                        q_block(h, NST - 1, h3_all)
                    # denominator: full pairwise tree-sum of the 16 exp tiles
                    # (L1 on the otherwise-idle Pool engine — GPSIMD can't
                    # touch PSUM but these are SBUF->SBUF; upper levels on
                    # DVE), then a SINGLE ones-matmul after the AV chain
                    # (partition-reduce + broadcast in one 512-col pass; the
                    # tree is done by then so the PE never waits on it)
                    lvl = pt_tiles
                    level = 0
                    while len(lvl) > 1:
                        nxt = []
                        for g in range(0, len(lvl), 2):
                            s = dsum_pool.tile([128, ST], D, name="ds", tag="ds")
                            # last two L1 pairs stay on DVE: a ~1.2us Pool add
                            # on the exp15 critical path delays the den matmul
                            eng = (
                                nc.gpsimd
                                if (level == 0 and g < 12)
                                else nc.vector
                            )
                            eng.tensor_add(s[:], lvl[g][:], lvl[g + 1][:])
                            nxt.append(s)
                        lvl = nxt
                        level += 1
                    psum_all = lvl[0]
                    ps_d = ps_den.tile([128, ST], F32, name="den", tag="den")
                    ps_o = ps_acc.tile([128, ST], F32, name="acc", tag="acc")
                    for tcn in range(NT):
                        nc.tensor.matmul(
                            ps_o[:], V_all[:, tcn, :], pt_tiles[tcn][:],
                            start=(tcn == 0), stop=(tcn == NT - 1),
                        )
                    nc.tensor.matmul(
                        ps_d[:], ones[:], psum_all[:], start=True, stop=True
                    )
                    # reciprocal as exp(-ln(x)) on ScalarE: the shared
                    # natural_log_exp_and_others act table holds both, and
                    # this is ~2x faster end-to-end than DVE's 4us
                    # InstReciprocal while freeing DVE for the CAST stream.
                    lnden = nrm_pool.tile([128, ST], F32, name="lnden", tag="lnden")
                    recip = nrm_pool.tile([128, ST], F32, name="recip", tag="recip")
                    nc.scalar.activation(
                        lnden[:], ps_d[:],
                        mybir.ActivationFunctionType.Ln, scale=1.0,
                    )
                    nc.scalar.activation(
                        recip[:], lnden[:],
                        mybir.ActivationFunctionType.Exp, scale=-1.0,
                    )
                    nc.vector.tensor_mul(aoT_all[:, h, ssl], ps_o[:], recip[:])

            # phase-C tail for the last s-tile
            for oc in range(E // 128):
                c_block(oc, NST - 1)

    _split_multi_wait_insts(nc)
    return nc


def run(inputs: dict, dtype_mode: str = "bf16", trace: bool = False):
    """Host-side shard + run + gather. inputs keyed as reference.setup_inputs()."""
    import ml_dtypes
    from concourse.bass_utils import run_bass_kernel_spmd

    hidden = np.asarray(inputs["hidden_states"], dtype=np.float32)
    Wq = np.asarray(inputs["Wq"], dtype=np.float32)
    Wk = np.asarray(inputs["Wk"], dtype=np.float32)
    Wv = np.asarray(inputs["Wv"], dtype=np.float32)
    Wo = np.asarray(inputs["Wo"], dtype=np.float32)

    np_d = ml_dtypes.bfloat16 if dtype_mode == "bf16" else np.float32
    # Pre-tile every tensor into the exact SBUF layout the kernel DMAs to,
    # so all device DMAs are large contiguous per-partition lines.
    hT = hidden[0].T.astype(np_d)  # [E, S]
    h4 = np.ascontiguousarray(
        hT.reshape(NE, 128, NST, ST).transpose(2, 1, 0, 3)
    )  # [NST, 128, NE, ST]

    in_maps = []
    for c in range(8):
        qsl = slice(c * DQ, (c + 1) * DQ)
        ksl = slice(c * HD, (c + 1) * HD)
        wqT = Wq[qsl, :].T.astype(np_d)  # [E, DQ]
        wkT = Wk[ksl, :].T.astype(np_d)  # [E, HD]
        wvT = Wv[ksl, :].T.astype(np_d)
        woT = Wo[:, qsl].T.astype(np_d)  # [DQ, E]
        in_maps.append(
            {
                "hT4": h4,
                "wq4": np.ascontiguousarray(
                    wqT.reshape(NE, 128, DQ).transpose(1, 0, 2)
                ),
                "wk4": np.ascontiguousarray(
                    wkT.reshape(NE, 128, HD).transpose(1, 0, 2)
                ),
                "wv4": np.ascontiguousarray(
                    wvT.reshape(NE, 128, HD).transpose(1, 0, 2)
                ),
                "wo4": np.ascontiguousarray(
                    woT.reshape(H, 128, E).transpose(1, 0, 2)
                ),
            }
        )

    nc = build(dtype_mode)
    res = run_bass_kernel_spmd(nc, in_maps, list(range(8)), trace=trace)
    acc = np.zeros((E // 128, 128, NST, ST), dtype=np.float32)
    for c in range(8):
        # out4: [E//128, NST, 128, ST] -> [E//128, 128, NST, ST]
        acc += np.asarray(res.results[c]["out4"], dtype=np.float32).transpose(
            0, 2, 1, 3
        )
    out = np.ascontiguousarray(acc.reshape(E, S).T)[None]  # [1, S, E]
    return out, res


# ---------------------------------------------------------------------------
# Self-contained harness entry point: full inputs in, full output out.
# Shards across the 8 NeuronCores tensor-parallel over heads (4 Q heads +
# their shared KV head per core); per-core o_proj partials summed on host.
# ---------------------------------------------------------------------------
DTYPE_MODE = "bf16"


def kernel(hidden_states, Wq, Wk, Wv, Wo):
    inputs = {
        "hidden_states": hidden_states,
        "Wq": Wq,
        "Wk": Wk,
        "Wv": Wv,
        "Wo": Wo,
    }
    out, _res = run(inputs, dtype_mode=DTYPE_MODE, trace=False)
    return out.astype(np.float32)



# revision 39
# speedup vs baseline: 1.0309x; 1.0309x over previous
"""GQA kernel builder for TRN2 (8-core tensor-parallel over heads).

Per core: 4 Q heads (128-dim each) + the 1 KV head they share.
All activations are kept feature-major ([feat_part, seq_free]) so every
matmul contracts over the partition dim:

  kT[d, t]  = sum_e WkT[e, d] * hT[e, t]          (lhsT=WkT tile, rhs=hT tile)
  V[t, d]   = sum_e hT[e, t] * WvT[e, d]          (lhsT=hT tile, rhs=WvT tile)
  qT[d, s]  = sum_e WqT[e, d] * hT[e, s]          (lhsT=WqT tile, rhs=hT tile)
  S^T[t, s] = sum_d kT[d, t] * qT[d, s]           (single matmul, d=128)
  P^T       = exp(S^T / sqrt(128))                (ScalarE, no max-subtract:
                                                   |scores| <~ 8 here)
  O^T[d, s] = sum_t V[t, d] * P^T[t, s]           (accumulate over t chunks)
  den[s]    = sum_t P^T[t, s]                     (ones-matmul, M=128 so the
                                                   sum lands broadcast)
  ao^T      = O^T * (1/den)                       (DVE reciprocal + mul)
  outT[o,s] = sum_f WoT[f, o] * ao^T[f, s]        (partial; host sums cores)

Structure (all bf16, f32 PSUM accumulate; ~468us HW, ~4x over the
f32 per-phase baseline):
- Few, large multi-dim DMAs (a dma_start costs ~0.6us of serial SP
  sequencer issue time), ordered by consumption so the first matmuls
  start ~15us in.
- Phase A streams hT once (2-deep s-tile double buffer): K, V, Q per
  s-tile. Q(st3) is held back; its hT tile persists.
- Phase B runs st-outer and interleaves PE filler between the exp
  (ScalarE) and the AV matmuls of each iteration: phase-C o_proj
  blocks of the previous s-tile (or the held-back Q(st3) blocks for
  st0). The softmax denominator is quad-summed on DVE and
  partition-reduced by 4 accumulating ones-matmuls.
- o_proj partials are written as bf16 and summed on the host.
"""

import math
import numpy as np
from contextlib import ExitStack

import concourse.bass as bass
import concourse.mybir as mybir
import concourse.tile as tile
from concourse.vector_clock import ScopedClock

F32 = mybir.dt.float32
BF16 = mybir.dt.bfloat16

S = 2048
E = 4096
HD = 128
H = 4          # Q heads per core
DQ = H * HD    # 512
ST = 512       # seq tile (free dim of most matmuls)
NST = S // ST  # 4
NE = E // 128  # 32
NT = S // 128  # 16
SCALE = 1.0 / math.sqrt(128.0)

MAX_DRAIN_WAITS = 1


class SplitDrainTileContext(tile.TileContext):
    """Walrus CoreV3 rejects >1 sync wait on an instruction; TileContext's
    exit attaches the whole residual vector clock to one Drain. Split it
    into a chain of Drains (SP executes them in order — equivalent)."""

    def _drain_and_barrier(self, tick_clock, wait_clock):
        drain_inst = self.nc.sync.drain()
        wait_clock.add_sem_waits(
            drain_inst.ins, ScopedClock({None: tick_clock.global_clock})
        )
        si = drain_inst.ins.sync_info
        waits = list(si.on_wait) if si is not None and si.on_wait else []
        if len(waits) > MAX_DRAIN_WAITS:
            si.on_wait = waits[:MAX_DRAIN_WAITS]
            rest = waits[MAX_DRAIN_WAITS:]
            for i in range(0, len(rest), MAX_DRAIN_WAITS):
                extra = self.nc.sync.drain()
                extra.ins.sync_info = mybir.SyncInfo(
                    on_wait=rest[i : i + MAX_DRAIN_WAITS], on_update=[]
                )

        self.nc.all_engine_barrier()
        assert self.sems is not None
        popped = self.nc._tile_sem_poison_stack.pop()
        assert popped is self._sem_poison
        self.nc.clear_and_free_semaphores(list(self.sems.allocated().values()))
        self.nc.all_engine_barrier()


def _split_multi_wait_insts(nc, max_waits: int = 1):
    """Walrus CoreV2/V3 codegen rejects instructions with more than one sync
    wait command. Hoist excess waits onto preceding same-engine NoOps — the
    engine executes them in order, so the gating is equivalent (for DMA the
    issuing sequencer stalls instead of the DGE queue: conservative, safe)."""
    for fn in nc.m.functions:
        for blk in fn.blocks:
            out = []
            for inst in blk.instructions:
                si = inst.sync_info
                waits = list(si.on_wait) if si is not None and si.on_wait else []
                if len(waits) > max_waits:
                    excess, keep = waits[:-max_waits], waits[-max_waits:]
                    for j, w in enumerate(excess):
                        nop = mybir.InstNoOp(name=f"{inst.name}-sw{j}")
                        nop.engine = inst.engine
                        nop.sync_info = mybir.SyncInfo(on_wait=[w], on_update=[])
                        out.append(nop)
                    si.on_wait = keep
                out.append(inst)
            blk.instructions = out


def build(dtype_mode: str = "bf16") -> bass.Bass:
    """dtype_mode: 'f32' | 'bf16'"""
    D = BF16 if dtype_mode == "bf16" else F32
    OUT_D = BF16 if dtype_mode == "bf16" else F32

    nc = bass.Bass()
    # All DRAM params are host-pre-tiled to the exact SBUF layout so every
    # DMA reads/writes large contiguous per-partition lines (the flat [E,S]
    # layout only reached ~175GB/s on the strided 1KB-line loads).
    hT4 = nc.declare_dram_parameter("hT4", [NST, 128, NE, ST], D, isOutput=False)
    wq4 = nc.declare_dram_parameter("wq4", [128, NE, DQ], D, isOutput=False)
    wk4 = nc.declare_dram_parameter("wk4", [128, NE, HD], D, isOutput=False)
    wv4 = nc.declare_dram_parameter("wv4", [128, NE, HD], D, isOutput=False)
    wo4 = nc.declare_dram_parameter("wo4", [128, H, E], D, isOutput=False)
    out4 = nc.declare_dram_parameter(
        "out4", [E // 128, NST, 128, ST], OUT_D, isOutput=True
    )

    with SplitDrainTileContext(nc) as tc, ExitStack() as octx:
        persist = octx.enter_context(tc.tile_pool(name="persist", bufs=1))
        weights = octx.enter_context(tc.tile_pool(name="weights", bufs=1))
        h3_pool = octx.enter_context(tc.tile_pool(name="h3", bufs=1))
        ps_mm = octx.enter_context(tc.tile_pool(name="ps_mm", bufs=5, space="PSUM"))
        ps_acc = octx.enter_context(tc.tile_pool(name="ps_acc", bufs=2, space="PSUM"))
        ps_den = octx.enter_context(tc.tile_pool(name="ps_den", bufs=1, space="PSUM"))

        # Persistent activations (merged into few tiles: each tile slot costs
        # semaphores that are individually zeroed in the NEFF epilogue)
        qT_all = persist.tile([128, H, S], D, name="qT", tag="qT")
        kT_t = persist.tile([128, S], D, name="kT", tag="kT")
        V_all = persist.tile([128, NT, HD], D, name="V", tag="V")
        # All-ones stationary operand: the denominator matmul uses M=128 so
        # the row-sum lands broadcast across all 128 PSUM partitions (same
        # N-cycle streaming cost as M=1, and DVE can then consume it without
        # a partition-broadcast).
        ones = persist.tile([128, 128], D, name="ones", tag="ones")
        nc.vector.memset(ones[:], 1.0)
        # Warm the Exp activation table during the DMA-bound head: the lazy
        # ACT_TABLE_LOAD otherwise costs ~1.3us at the first phase-B exp,
        # stalling the PE at the A->B transition.
        warm = persist.tile([128, 1], D, name="warm", tag="warm")
        nc.scalar.activation(
            warm[:], ones[:, :1], mybir.ActivationFunctionType.Exp, scale=SCALE
        )

        wq_all = weights.tile([128, NE, DQ], D, name="wq", tag="wq")
        # h for the last s-tile stays resident past phase A: its Q blocks
        # are held back to fill the PE during B(st0) (which has no phase-C
        # filler), so the tile must outlive the phase-A h pool.
        h3_all = h3_pool.tile([128, NE, ST], D, name="h3", tag="h3")

        def q_block(dq, st, h_src):
            ssl = slice(st * ST, (st + 1) * ST)
            ps = ps_mm.tile([128, ST], F32, name="mm", tag="mm")
            for e in range(NE):
                nc.tensor.matmul(
                    ps[:],
                    wq_all[:, e, dq * 128 : (dq + 1) * 128],
                    h_src[:, e, :],
                    start=(e == 0), stop=(e == NE - 1),
                )
            nc.vector.tensor_copy(qT_all[:, dq, ssl], ps[:])

        # ---- Phase A (fused): one pass over hT computes K, V, Q.
        # DMAs are few and big (each dma_start costs ~0.6us of serial issue
        # time on the SP sequencer) and issued in consumption order:
        # wk -> hT(st0) -> wv -> hT(st1) -> wq -> hT(st2) -> hT(st3).
        with ExitStack() as actx:
            wkv_pool = actx.enter_context(tc.tile_pool(name="wkv", bufs=1))
            hA_pool = actx.enter_context(tc.tile_pool(name="hA", bufs=2))
            wk_all = wkv_pool.tile([128, NE, HD], D, name="wk", tag="wk")
            wv_all = wkv_pool.tile([128, NE, HD], D, name="wv", tag="wv")
            # Single serial DMA stream on Sync in exact consumption order:
            # splitting across the two HWDGE queues was tried and lost —
            # concurrent queues split the (slow, ~200GB/s early-window)
            # bandwidth and the late consumers' loads finish LATER than a
            # serial consumption-ordered stream.
            nc.sync.dma_start(wk_all[:, : NE // 8, :], wk4[:, : NE // 8, :])
            for st in range(NST):
                ssl = slice(st * ST, (st + 1) * ST)
                if st == NST - 1:
                    h_st = h3_all
                else:
                    h_st = hA_pool.tile([128, NE, ST], D, name="h", tag="h")
                if st == 0:
                    nc.sync.dma_start(
                        h_st[:, : NE // 8, :], hT4[0, :, : NE // 8, :]
                    )
                    nc.sync.dma_start(wk_all[:, NE // 8 :, :], wk4[:, NE // 8 :, :])
                    nc.sync.dma_start(
                        h_st[:, NE // 8 : NE // 4, :],
                        hT4[0, :, NE // 8 : NE // 4, :],
                    )
                    for qtr in range(1, 4):
                        nc.sync.dma_start(
                            h_st[:, qtr * (NE // 4) : (qtr + 1) * (NE // 4), :],
                            hT4[0, :, qtr * (NE // 4) : (qtr + 1) * (NE // 4), :],
                        )
                        if qtr == 1:
                            nc.sync.dma_start(wv_all[:], wv4[:, :, :])
                    nc.sync.dma_start(
                        wq_all[:, :, : DQ // 2], wq4[:, :, : DQ // 2]
                    )
                    nc.sync.dma_start(
                        wq_all[:, :, DQ // 2 :], wq4[:, :, DQ // 2 :]
                    )
                else:
                    nc.sync.dma_start(h_st[:], hT4[st, :, :, :])
                # K projection
                ps = ps_mm.tile([128, ST], F32, name="mm", tag="mm")
                for e in range(NE):
                    nc.tensor.matmul(
                        ps[:], wk_all[:, e, :], h_st[:, e, :],
                        start=(e == 0), stop=(e == NE - 1),
                    )
                nc.vector.tensor_copy(kT_t[:, ssl], ps[:])
                # V projection (natural [t, d] layout)
                for tc4 in range(ST // 128):
                    tglob = st * (ST // 128) + tc4
                    ps = ps_mm.tile([128, HD], F32, name="mm", tag="mm")
                    for e in range(NE):
                        nc.tensor.matmul(
                            ps[:],
                            h_st[:, e, tc4 * 128 : (tc4 + 1) * 128],
                            wv_all[:, e, :],
                            start=(e == 0), stop=(e == NE - 1),
                        )
                    nc.vector.tensor_copy(V_all[:, tglob, :], ps[:])
                # Q projection (st3 blocks held back as B(st0) filler)
                if st < NST - 1:
                    for dq in range(H):
                        q_block(dq, st, h_st)

        # ---- Phases B+C interleaved: B runs st-outer so phase-C blocks of
        # the previous s-tile can fill the PE while ScalarE runs the exps.
        # The softmax denominator is quad-summed on DVE (12 adds) and
        # partition-reduced by 4 accumulating ones-matmuls (PE cost ~0.9us
        # per iteration instead of 3.5us for 16 ones-matmuls).
        with ExitStack() as bctx:
            pt_pool = bctx.enter_context(tc.tile_pool(name="pt", bufs=20))
            dsum_pool = bctx.enter_context(tc.tile_pool(name="dsum", bufs=10))
            nrm_pool = bctx.enter_context(tc.tile_pool(name="nrm", bufs=2))
            ao_pool = bctx.enter_context(tc.tile_pool(name="ao", bufs=1))
            stg_pool = bctx.enter_context(tc.tile_pool(name="stg", bufs=6))
            wo_all = weights.tile([128, H, E], D, name="wo", tag="wo")
            aoT_all = ao_pool.tile([128, H, S], D, name="ao", tag="ao")

            def c_block(oc, st):
                ssl = slice(st * ST, (st + 1) * ST)
                ps = ps_mm.tile([128, ST], F32, name="mm", tag="mm")
                for fc in range(H):
                    nc.tensor.matmul(
                        ps[:],
                        wo_all[:, fc, oc * 128 : (oc + 1) * 128],
                        aoT_all[:, fc, ssl],
                        start=(fc == 0), stop=(fc == H - 1),
                    )
                stg = stg_pool.tile([128, ST], OUT_D, name="stg", tag="stg")
                nc.vector.tensor_copy(stg[:], ps[:])
                nc.sync.dma_start(out4[oc, st, :, :], stg[:])

            for st in range(NST):
                ssl = slice(st * ST, (st + 1) * ST)
                for h in range(H):
                    # scores + exp
                    pt_tiles = []
                    for tcn in range(NT):
                        ps = ps_mm.tile([128, ST], F32, name="mm", tag="mm")
                        nc.tensor.matmul(
                            ps[:],
                            kT_t[:, tcn * 128 : (tcn + 1) * 128],
                            qT_all[:, h, ssl],
                            start=True, stop=True,
                        )
                        pt = pt_pool.tile([128, ST], D, name="pt", tag="pt")
                        nc.scalar.activation(
                            pt[:], ps[:], mybir.ActivationFunctionType.Exp,
                            scale=SCALE,
                        )
                        pt_tiles.append(pt)
                    if st == 0 and h == 0:
                        nc.sync.dma_start(wo_all[:], wo4[:, :, :])
                    # filler to keep the PE busy while this iteration's
                    # exps run on ScalarE: phase-C blocks of the previous
                    # s-tile, or (for st0) the held-back Q(st3) blocks
                    if st > 0:
                        for oc in range(h * 8, h * 8 + 8):
                            c_block(oc, st - 1)
                    else:
                        q_block(h, NST - 1, h3_all)
                    # denominator: full pairwise tree-sum of the 16 exp tiles
                    # (L1 on the otherwise-idle Pool engine — GPSIMD can't
                    # touch PSUM but these are SBUF->SBUF; upper levels on
                    # DVE), then a SINGLE ones-matmul after the AV chain
                    # (partition-reduce + broadcast in one 512-col pass; the
                    # tree is done by then so the PE never waits on it)
                    lvl = pt_tiles
                    level = 0
                    while len(lvl) > 1:
                        nxt = []
                        for g in range(0, len(lvl), 2):
                            s = dsum_pool.tile([128, ST], D, name="ds", tag="ds")
                            nc.vector.tensor_add(s[:], lvl[g][:], lvl[g + 1][:])
                            nxt.append(s)
                        lvl = nxt
                        level += 1
                    psum_all = lvl[0]
                    ps_d = ps_den.tile([128, ST], F32, name="den", tag="den")
                    ps_o = ps_acc.tile([128, ST], F32, name="acc", tag="acc")
                    for tcn in range(NT):
                        nc.tensor.matmul(
                            ps_o[:], V_all[:, tcn, :], pt_tiles[tcn][:],
                            start=(tcn == 0), stop=(tcn == NT - 1),
                        )
                    nc.tensor.matmul(
                        ps_d[:], ones[:], psum_all[:], start=True, stop=True
                    )
                    # (Tried alternatives that LOST: exp(-ln(x)) on ScalarE
                    # queues behind the next iteration's exp stream (+1.2us/
                    # iter chain delay); DVE has no hw divide; custom-DVE
                    # reciprocal_approx_fast fails CoreV2 walrus codegen.)
                    recip = nrm_pool.tile([128, ST], F32, name="recip", tag="recip")
                    nc.vector.reciprocal(recip[:], ps_d[:])
                    nc.vector.tensor_mul(aoT_all[:, h, ssl], ps_o[:], recip[:])

            # phase-C tail for the last s-tile
            for oc in range(E // 128):
                c_block(oc, NST - 1)

    _split_multi_wait_insts(nc)
    return nc


def run(inputs: dict, dtype_mode: str = "bf16", trace: bool = False):
    """Host-side shard + run + gather. inputs keyed as reference.setup_inputs()."""
    import ml_dtypes
    from concourse.bass_utils import run_bass_kernel_spmd

    hidden = np.asarray(inputs["hidden_states"], dtype=np.float32)
    Wq = np.asarray(inputs["Wq"], dtype=np.float32)
    Wk = np.asarray(inputs["Wk"], dtype=np.float32)
    Wv = np.asarray(inputs["Wv"], dtype=np.float32)
    Wo = np.asarray(inputs["Wo"], dtype=np.float32)

    np_d = ml_dtypes.bfloat16 if dtype_mode == "bf16" else np.float32
    # Pre-tile every tensor into the exact SBUF layout the kernel DMAs to,
    # so all device DMAs are large contiguous per-partition lines.
    hT = hidden[0].T.astype(np_d)  # [E, S]
    h4 = np.ascontiguousarray(
        hT.reshape(NE, 128, NST, ST).transpose(2, 1, 0, 3)
    )  # [NST, 128, NE, ST]

    in_maps = []
    for c in range(8):
        qsl = slice(c * DQ, (c + 1) * DQ)
        ksl = slice(c * HD, (c + 1) * HD)
        wqT = Wq[qsl, :].T.astype(np_d)  # [E, DQ]
        wkT = Wk[ksl, :].T.astype(np_d)  # [E, HD]
        wvT = Wv[ksl, :].T.astype(np_d)
        woT = Wo[:, qsl].T.astype(np_d)  # [DQ, E]
        in_maps.append(
            {
                "hT4": h4,
                "wq4": np.ascontiguousarray(
                    wqT.reshape(NE, 128, DQ).transpose(1, 0, 2)
                ),
                "wk4": np.ascontiguousarray(
                    wkT.reshape(NE, 128, HD).transpose(1, 0, 2)
                ),
                "wv4": np.ascontiguousarray(
                    wvT.reshape(NE, 128, HD).transpose(1, 0, 2)
                ),
                "wo4": np.ascontiguousarray(
                    woT.reshape(H, 128, E).transpose(1, 0, 2)
                ),
            }
        )

    nc = build(dtype_mode)
    res = run_bass_kernel_spmd(nc, in_maps, list(range(8)), trace=trace)
    acc = np.zeros((E // 128, 128, NST, ST), dtype=np.float32)
    for c in range(8):
        # out4: [E//128, NST, 128, ST] -> [E//128, 128, NST, ST]
        acc += np.asarray(res.results[c]["out4"], dtype=np.float32).transpose(
            0, 2, 1, 3
        )
    out = np.ascontiguousarray(acc.reshape(E, S).T)[None]  # [1, S, E]
    return out, res


# ---------------------------------------------------------------------------
# Self-contained harness entry point: full inputs in, full output out.
# Shards across the 8 NeuronCores tensor-parallel over heads (4 Q heads +
# their shared KV head per core); per-core o_proj partials summed on host.
# ---------------------------------------------------------------------------
DTYPE_MODE = "bf16"


def kernel(hidden_states, Wq, Wk, Wv, Wo):
    inputs = {
        "hidden_states": hidden_states,
        "Wq": Wq,
        "Wk": Wk,
        "Wv": Wv,
        "Wo": Wo,
    }
    out, _res = run(inputs, dtype_mode=DTYPE_MODE, trace=False)
    return out.astype(np.float32)



# revision 41
# speedup vs baseline: 1.0766x; 1.0444x over previous
"""GQA kernel builder for TRN2 (8-core tensor-parallel over heads).

Per core: 4 Q heads (128-dim each) + the 1 KV head they share.
All activations are kept feature-major ([feat_part, seq_free]) so every
matmul contracts over the partition dim:

  kT[d, t]  = sum_e WkT[e, d] * hT[e, t]          (lhsT=WkT tile, rhs=hT tile)
  V[t, d]   = sum_e hT[e, t] * WvT[e, d]          (lhsT=hT tile, rhs=WvT tile)
  qT[d, s]  = sum_e WqT[e, d] * hT[e, s]          (lhsT=WqT tile, rhs=hT tile)
  S^T[t, s] = sum_d kT[d, t] * qT[d, s]           (single matmul, d=128)
  P^T       = exp(S^T / sqrt(128))                (ScalarE, no max-subtract:
                                                   |scores| <~ 8 here)
  O^T[d, s] = sum_t V[t, d] * P^T[t, s]           (accumulate over t chunks)
  den[s]    = sum_t P^T[t, s]                     (ones-matmul, M=128 so the
                                                   sum lands broadcast)
  ao^T      = O^T * (1/den)                       (DVE reciprocal + mul)
  outT[o,s] = sum_f WoT[f, o] * ao^T[f, s]        (partial; host sums cores)

Structure (all bf16, f32 PSUM accumulate; ~468us HW, ~4x over the
f32 per-phase baseline):
- Few, large multi-dim DMAs (a dma_start costs ~0.6us of serial SP
  sequencer issue time), ordered by consumption so the first matmuls
  start ~15us in.
- Phase A streams hT once (2-deep s-tile double buffer): K, V, Q per
  s-tile. Q(st3) is held back; its hT tile persists.
- Phase B runs st-outer and interleaves PE filler between the exp
  (ScalarE) and the AV matmuls of each iteration: phase-C o_proj
  blocks of the previous s-tile (or the held-back Q(st3) blocks for
  st0). The softmax denominator is quad-summed on DVE and
  partition-reduced by 4 accumulating ones-matmuls.
- o_proj partials are written as bf16 and summed on the host.
"""

import math
import numpy as np
from contextlib import ExitStack

import concourse.bass as bass
import concourse.mybir as mybir
import concourse.tile as tile
from concourse.vector_clock import ScopedClock

F32 = mybir.dt.float32
BF16 = mybir.dt.bfloat16

S = 2048
E = 4096
HD = 128
H = 4          # Q heads per core
DQ = H * HD    # 512
ST = 512       # seq tile (free dim of most matmuls)
NST = S // ST  # 4
NE = E // 128  # 32
NT = S // 128  # 16
SCALE = 1.0 / math.sqrt(128.0)

MAX_DRAIN_WAITS = 1


class SplitDrainTileContext(tile.TileContext):
    """Walrus CoreV3 rejects >1 sync wait on an instruction; TileContext's
    exit attaches the whole residual vector clock to one Drain. Split it
    into a chain of Drains (SP executes them in order — equivalent)."""

    def _drain_and_barrier(self, tick_clock, wait_clock):
        drain_inst = self.nc.sync.drain()
        wait_clock.add_sem_waits(
            drain_inst.ins, ScopedClock({None: tick_clock.global_clock})
        )
        si = drain_inst.ins.sync_info
        waits = list(si.on_wait) if si is not None and si.on_wait else []
        if len(waits) > MAX_DRAIN_WAITS:
            si.on_wait = waits[:MAX_DRAIN_WAITS]
            rest = waits[MAX_DRAIN_WAITS:]
            for i in range(0, len(rest), MAX_DRAIN_WAITS):
                extra = self.nc.sync.drain()
                extra.ins.sync_info = mybir.SyncInfo(
                    on_wait=rest[i : i + MAX_DRAIN_WAITS], on_update=[]
                )

        self.nc.all_engine_barrier()
        assert self.sems is not None
        popped = self.nc._tile_sem_poison_stack.pop()
        assert popped is self._sem_poison
        self.nc.clear_and_free_semaphores(list(self.sems.allocated().values()))
        self.nc.all_engine_barrier()


def _split_multi_wait_insts(nc, max_waits: int = 1):
    """Walrus CoreV2/V3 codegen rejects instructions with more than one sync
    wait command. Hoist excess waits onto preceding same-engine NoOps — the
    engine executes them in order, so the gating is equivalent (for DMA the
    issuing sequencer stalls instead of the DGE queue: conservative, safe)."""
    for fn in nc.m.functions:
        for blk in fn.blocks:
            out = []
            for inst in blk.instructions:
                si = inst.sync_info
                waits = list(si.on_wait) if si is not None and si.on_wait else []
                if len(waits) > max_waits:
                    excess, keep = waits[:-max_waits], waits[-max_waits:]
                    for j, w in enumerate(excess):
                        nop = mybir.InstNoOp(name=f"{inst.name}-sw{j}")
                        nop.engine = inst.engine
                        nop.sync_info = mybir.SyncInfo(on_wait=[w], on_update=[])
                        out.append(nop)
                    si.on_wait = keep
                out.append(inst)
            blk.instructions = out


def build(dtype_mode: str = "bf16") -> bass.Bass:
    """dtype_mode: 'f32' | 'bf16'"""
    D = BF16 if dtype_mode == "bf16" else F32
    OUT_D = BF16 if dtype_mode == "bf16" else F32

    nc = bass.Bass()
    # All DRAM params are host-pre-tiled to the exact SBUF layout so every
    # DMA reads/writes large contiguous per-partition lines (the flat [E,S]
    # layout only reached ~175GB/s on the strided 1KB-line loads).
    hT4 = nc.declare_dram_parameter("hT4", [NST, 128, NE, ST], D, isOutput=False)
    wq4 = nc.declare_dram_parameter("wq4", [128, NE, DQ], D, isOutput=False)
    wk4 = nc.declare_dram_parameter("wk4", [128, NE, HD], D, isOutput=False)
    wv4 = nc.declare_dram_parameter("wv4", [128, NE, HD], D, isOutput=False)
    wo4 = nc.declare_dram_parameter("wo4", [128, H, E], D, isOutput=False)
    out4 = nc.declare_dram_parameter(
        "out4", [E // 128, NST, 128, ST], OUT_D, isOutput=True
    )

    with SplitDrainTileContext(nc) as tc, ExitStack() as octx:
        persist = octx.enter_context(tc.tile_pool(name="persist", bufs=1))
        weights = octx.enter_context(tc.tile_pool(name="weights", bufs=1))
        h3_pool = octx.enter_context(tc.tile_pool(name="h3", bufs=1))
        ps_mm = octx.enter_context(tc.tile_pool(name="ps_mm", bufs=5, space="PSUM"))
        ps_acc = octx.enter_context(tc.tile_pool(name="ps_acc", bufs=2, space="PSUM"))
        ps_den = octx.enter_context(tc.tile_pool(name="ps_den", bufs=1, space="PSUM"))

        # Persistent activations (merged into few tiles: each tile slot costs
        # semaphores that are individually zeroed in the NEFF epilogue)
        qT_all = persist.tile([128, H, S], D, name="qT", tag="qT")
        kT_t = persist.tile([128, S], D, name="kT", tag="kT")
        V_all = persist.tile([128, NT, HD], D, name="V", tag="V")
        # All-ones stationary operand: the denominator matmul uses M=128 so
        # the row-sum lands broadcast across all 128 PSUM partitions (same
        # N-cycle streaming cost as M=1, and DVE can then consume it without
        # a partition-broadcast).
        ones = persist.tile([128, 128], D, name="ones", tag="ones")
        nc.vector.memset(ones[:], 1.0)
        # Warm the Exp activation table during the DMA-bound head: the lazy
        # ACT_TABLE_LOAD otherwise costs ~1.3us at the first phase-B exp,
        # stalling the PE at the A->B transition.
        warm = persist.tile([128, 1], D, name="warm", tag="warm")
        nc.scalar.activation(
            warm[:], ones[:, :1], mybir.ActivationFunctionType.Exp, scale=SCALE
        )

        wq_all = weights.tile([128, NE, DQ], D, name="wq", tag="wq")
        # h for the last s-tile stays resident past phase A: its Q blocks
        # are held back to fill the PE during B(st0) (which has no phase-C
        # filler), so the tile must outlive the phase-A h pool.
        h3_all = h3_pool.tile([128, NE, ST], D, name="h3", tag="h3")

        def q_block(dq, st, h_src):
            ssl = slice(st * ST, (st + 1) * ST)
            ps = ps_mm.tile([128, ST], F32, name="mm", tag="mm")
            for e in range(NE):
                nc.tensor.matmul(
                    ps[:],
                    wq_all[:, e, dq * 128 : (dq + 1) * 128],
                    h_src[:, e, :],
                    start=(e == 0), stop=(e == NE - 1),
                )
            nc.vector.tensor_copy(qT_all[:, dq, ssl], ps[:])

        # ---- Phase A (fused): one pass over hT computes K, V, Q.
        # DMAs are few and big (each dma_start costs ~0.6us of serial issue
        # time on the SP sequencer) and issued in consumption order:
        # wk -> hT(st0) -> wv -> hT(st1) -> wq -> hT(st2) -> hT(st3).
        with ExitStack() as actx:
            wkv_pool = actx.enter_context(tc.tile_pool(name="wkv", bufs=1))
            hA_pool = actx.enter_context(tc.tile_pool(name="hA", bufs=2))
            wk_all = wkv_pool.tile([128, NE, HD], D, name="wk", tag="wk")
            wv_all = wkv_pool.tile([128, NE, HD], D, name="wv", tag="wv")
            # Single serial DMA stream on Sync in exact consumption order:
            # splitting across the two HWDGE queues was tried and lost —
            # concurrent queues split the (slow, ~200GB/s early-window)
            # bandwidth and the late consumers' loads finish LATER than a
            # serial consumption-ordered stream.
            nc.sync.dma_start(wk_all[:, : NE // 8, :], wk4[:, : NE // 8, :])
            for st in range(NST):
                ssl = slice(st * ST, (st + 1) * ST)
                if st == NST - 1:
                    h_st = h3_all
                else:
                    h_st = hA_pool.tile([128, NE, ST], D, name="h", tag="h")
                if st == 0:
                    nc.sync.dma_start(
                        h_st[:, : NE // 8, :], hT4[0, :, : NE // 8, :]
                    )
                    nc.sync.dma_start(wk_all[:, NE // 8 :, :], wk4[:, NE // 8 :, :])
                    nc.sync.dma_start(
                        h_st[:, NE // 8 : NE // 4, :],
                        hT4[0, :, NE // 8 : NE // 4, :],
                    )
                    for qtr in range(1, 4):
                        nc.sync.dma_start(
                            h_st[:, qtr * (NE // 4) : (qtr + 1) * (NE // 4), :],
                            hT4[0, :, qtr * (NE // 4) : (qtr + 1) * (NE // 4), :],
                        )
                        if qtr == 1:
                            nc.sync.dma_start(wv_all[:], wv4[:, :, :])
                    nc.sync.dma_start(
                        wq_all[:, :, : DQ // 2], wq4[:, :, : DQ // 2]
                    )
                    nc.sync.dma_start(
                        wq_all[:, :, DQ // 2 :], wq4[:, :, DQ // 2 :]
                    )
                else:
                    nc.sync.dma_start(h_st[:], hT4[st, :, :, :])
                # K projection
                ps = ps_mm.tile([128, ST], F32, name="mm", tag="mm")
                for e in range(NE):
                    nc.tensor.matmul(
                        ps[:], wk_all[:, e, :], h_st[:, e, :],
                        start=(e == 0), stop=(e == NE - 1),
                    )
                nc.vector.tensor_copy(kT_t[:, ssl], ps[:])
                # V projection (natural [t, d] layout)
                for tc4 in range(ST // 128):
                    tglob = st * (ST // 128) + tc4
                    ps = ps_mm.tile([128, HD], F32, name="mm", tag="mm")
                    for e in range(NE):
                        nc.tensor.matmul(
                            ps[:],
                            h_st[:, e, tc4 * 128 : (tc4 + 1) * 128],
                            wv_all[:, e, :],
                            start=(e == 0), stop=(e == NE - 1),
                        )
                    nc.vector.tensor_copy(V_all[:, tglob, :], ps[:])
                # Q projection (st3 blocks held back as B(st0) filler)
                if st < NST - 1:
                    for dq in range(H):
                        q_block(dq, st, h_st)

        # ---- Phases B+C interleaved: B runs st-outer so phase-C blocks of
        # the previous s-tile can fill the PE while ScalarE runs the exps.
        # The softmax denominator is quad-summed on DVE (12 adds) and
        # partition-reduced by 4 accumulating ones-matmuls (PE cost ~0.9us
        # per iteration instead of 3.5us for 16 ones-matmuls).
        with ExitStack() as bctx:
            pt_pool = bctx.enter_context(tc.tile_pool(name="pt", bufs=24))
            dsum_pool = bctx.enter_context(tc.tile_pool(name="dsum", bufs=10))
            nrm_pool = bctx.enter_context(tc.tile_pool(name="nrm", bufs=2))
            ao_pool = bctx.enter_context(tc.tile_pool(name="ao", bufs=1))
            stg_pool = bctx.enter_context(tc.tile_pool(name="stg", bufs=6))
            wo_all = weights.tile([128, H, E], D, name="wo", tag="wo")
            aoT_all = ao_pool.tile([128, H, S], D, name="ao", tag="ao")

            def c_block(oc, st):
                ssl = slice(st * ST, (st + 1) * ST)
                ps = ps_mm.tile([128, ST], F32, name="mm", tag="mm")
                for fc in range(H):
                    nc.tensor.matmul(
                        ps[:],
                        wo_all[:, fc, oc * 128 : (oc + 1) * 128],
                        aoT_all[:, fc, ssl],
                        start=(fc == 0), stop=(fc == H - 1),
                    )
                stg = stg_pool.tile([128, ST], OUT_D, name="stg", tag="stg")
                nc.vector.tensor_copy(stg[:], ps[:])
                nc.sync.dma_start(out4[oc, st, :, :], stg[:])

            iters = [(st, h) for st in range(NST) for h in range(H)]
            PRE = 5  # scores of iter i+1 prefetched before AV(i)

            def emit_scores(it_idx, tcn_lo, tcn_hi):
                st, h = iters[it_idx]
                issl = slice(st * ST, (st + 1) * ST)
                tiles = []
                for tcn in range(tcn_lo, tcn_hi):
                    ps = ps_mm.tile([128, ST], F32, name="mm", tag="mm")
                    nc.tensor.matmul(
                        ps[:],
                        kT_t[:, tcn * 128 : (tcn + 1) * 128],
                        qT_all[:, h, issl],
                        start=True, stop=True,
                    )
                    pt = pt_pool.tile([128, ST], D, name="pt", tag="pt")
                    nc.scalar.activation(
                        pt[:], ps[:], mybir.ActivationFunctionType.Exp,
                        scale=SCALE,
                    )
                    tiles.append(pt)
                return tiles

            prefetched = []
            for idx, (st, h) in enumerate(iters):
                ssl = slice(st * ST, (st + 1) * ST)
                pt_tiles = prefetched + emit_scores(idx, len(prefetched), NT)
                if st == 0 and h == 0:
                    nc.sync.dma_start(wo_all[:], wo4[:, :, :])
                # filler to keep the PE busy while this iteration's
                # exps run on ScalarE: phase-C blocks of the previous
                # s-tile, or (for st0) the held-back Q(st3) blocks
                if st > 0:
                    for oc in range(h * 8, h * 8 + 8):
                        c_block(oc, st - 1)
                else:
                    q_block(h, NST - 1, h3_all)
                # denominator: full pairwise tree-sum of the 16 exp tiles
                # on DVE, then a SINGLE ones-matmul after the AV chain
                # (partition-reduce + broadcast in one 512-col pass; the
                # tree is done by then so the PE never waits on it)
                lvl = pt_tiles
                while len(lvl) > 1:
                    nxt = []
                    for g in range(0, len(lvl), 2):
                        s = dsum_pool.tile([128, ST], D, name="ds", tag="ds")
                        nc.vector.tensor_add(s[:], lvl[g][:], lvl[g + 1][:])
                        nxt.append(s)
                    lvl = nxt
                psum_all = lvl[0]
                # software pipelining: the next iteration's first scores go
                # to the PE BEFORE this iteration's exp-gated AV tail, so
                # ScalarE's serial exp stream (the phase-B critical chain)
                # never waits on the AV/den handoff.
                if idx + 1 < len(iters):
                    prefetched = emit_scores(idx + 1, 0, PRE)
                else:
                    prefetched = []
                ps_d = ps_den.tile([128, ST], F32, name="den", tag="den")
                ps_o = ps_acc.tile([128, ST], F32, name="acc", tag="acc")
                for tcn in range(NT):
                    nc.tensor.matmul(
                        ps_o[:], V_all[:, tcn, :], pt_tiles[tcn][:],
                        start=(tcn == 0), stop=(tcn == NT - 1),
                    )
                nc.tensor.matmul(
                    ps_d[:], ones[:], psum_all[:], start=True, stop=True
                )
                # (Tried alternatives that LOST: exp(-ln(x)) on ScalarE
                # queues behind the next iteration's exp stream (+1.2us/
                # iter chain delay); DVE has no hw divide; custom-DVE
                # reciprocal_approx_fast fails CoreV2 walrus codegen.)
                recip = nrm_pool.tile([128, ST], F32, name="recip", tag="recip")
                nc.vector.reciprocal(recip[:], ps_d[:])
                nc.vector.tensor_mul(aoT_all[:, h, ssl], ps_o[:], recip[:])

            # phase-C tail for the last s-tile: the first blocks' fc0-2
            # partials run during the final reciprocal+normalize window and
            # only their fc3 matmuls wait on the last head's aoT.
            ssl3 = slice((NST - 1) * ST, NST * ST)
            tail_ps = []
            for oc in range(3):
                ps = ps_mm.tile([128, ST], F32, name="mm", tag="mm")
                for fc in range(3):
                    nc.tensor.matmul(
                        ps[:],
                        wo_all[:, fc, oc * 128 : (oc + 1) * 128],
                        aoT_all[:, fc, ssl3],
                        start=(fc == 0), stop=False,
                    )
                tail_ps.append(ps)
            for oc in range(3):
                nc.tensor.matmul(
                    tail_ps[oc][:],
                    wo_all[:, 3, oc * 128 : (oc + 1) * 128],
                    aoT_all[:, 3, ssl3],
                    start=False, stop=True,
                )
                stg = stg_pool.tile([128, ST], OUT_D, name="stg", tag="stg")
                nc.vector.tensor_copy(stg[:], tail_ps[oc][:])
                nc.sync.dma_start(out4[oc, NST - 1, :, :], stg[:])
            for oc in range(3, E // 128):
                c_block(oc, NST - 1)

    _split_multi_wait_insts(nc)
    return nc


def run(inputs: dict, dtype_mode: str = "bf16", trace: bool = False):
    """Host-side shard + run + gather. inputs keyed as reference.setup_inputs()."""
    import ml_dtypes
    from concourse.bass_utils import run_bass_kernel_spmd

    hidden = np.asarray(inputs["hidden_states"], dtype=np.float32)
    Wq = np.asarray(inputs["Wq"], dtype=np.float32)
    Wk = np.asarray(inputs["Wk"], dtype=np.float32)
    Wv = np.asarray(inputs["Wv"], dtype=np.float32)
    Wo = np.asarray(inputs["Wo"], dtype=np.float32)

    np_d = ml_dtypes.bfloat16 if dtype_mode == "bf16" else np.float32
    # Pre-tile every tensor into the exact SBUF layout the kernel DMAs to,
    # so all device DMAs are large contiguous per-partition lines.
    hT = hidden[0].T.astype(np_d)  # [E, S]
    h4 = np.ascontiguousarray(
        hT.reshape(NE, 128, NST, ST).transpose(2, 1, 0, 3)
    )  # [NST, 128, NE, ST]

    in_maps = []
    for c in range(8):
        qsl = slice(c * DQ, (c + 1) * DQ)
        ksl = slice(c * HD, (c + 1) * HD)
        wqT = Wq[qsl, :].T.astype(np_d)  # [E, DQ]
        wkT = Wk[ksl, :].T.astype(np_d)  # [E, HD]
        wvT = Wv[ksl, :].T.astype(np_d)
        woT = Wo[:, qsl].T.astype(np_d)  # [DQ, E]
        in_maps.append(
            {
                "hT4": h4,
                "wq4": np.ascontiguousarray(
                    wqT.reshape(NE, 128, DQ).transpose(1, 0, 2)
                ),
                "wk4": np.ascontiguousarray(
                    wkT.reshape(NE, 128, HD).transpose(1, 0, 2)
                ),
                "wv4": np.ascontiguousarray(
                    wvT.reshape(NE, 128, HD).transpose(1, 0, 2)
                ),
                "wo4": np.ascontiguousarray(
                    woT.reshape(H, 128, E).transpose(1, 0, 2)
                ),
            }
        )

    nc = build(dtype_mode)
    res = run_bass_kernel_spmd(nc, in_maps, list(range(8)), trace=trace)
    acc = np.zeros((E // 128, 128, NST, ST), dtype=np.float32)
    for c in range(8):
        # out4: [E//128, NST, 128, ST] -> [E//128, 128, NST, ST]
        acc += np.asarray(res.results[c]["out4"], dtype=np.float32).transpose(
            0, 2, 1, 3
        )
    out = np.ascontiguousarray(acc.reshape(E, S).T)[None]  # [1, S, E]
    return out, res


# ---------------------------------------------------------------------------
# Self-contained harness entry point: full inputs in, full output out.
# Shards across the 8 NeuronCores tensor-parallel over heads (4 Q heads +
# their shared KV head per core); per-core o_proj partials summed on host.
# ---------------------------------------------------------------------------
DTYPE_MODE = "bf16"


def kernel(hidden_states, Wq, Wk, Wv, Wo):
    inputs = {
        "hidden_states": hidden_states,
        "Wq": Wq,
        "Wk": Wk,
        "Wv": Wv,
        "Wo": Wo,
    }
    out, _res = run(inputs, dtype_mode=DTYPE_MODE, trace=False)
    return out.astype(np.float32)

